# revision 18
# baseline (speedup 1.0000x reference)
"""Causal self-attention with RoPE, sharded over 8 TRN2 NeuronCores.

Sharding: core = (batch b, head-group hg). Cores 0-3 -> batch 0, cores 4-7 ->
batch 1; head-group hg = core % 4 owns heads [3*hg, 3*hg+3). Each core computes
its heads' attention and a partial output projection (w_proj column-slice);
the host sums the 4 partials per batch (the row-sharded projection's
all-reduce, done on host since full outputs are gathered anyway).

Per-core kernel, bf16 data / fp32 PSUM accumulation:
  - QKV q/k features packed in 3x128 tiles [q0|q1] [k0|k1] [q2|k2];
    out = wqkT.T @ xT per 512-token block. k2 mirrored into a 4th slot's
    low half (SBUF->SBUF DMA) so head-2 scores see equal base partitions.
  - RoPE with signs folded into a permuted sin table: q' = (acc*cos) +
    P2T @ (acc*gsin) where P2T is the plain half-swap permutation and
    gsin = [+sin; -sin] per 32-half. Two muls off PSUM (DVE/Pool), one
    rotate matmul (lagged one acc-group so the PE never waits), one DVE add.
  - V^T computed directly: vt[t,d] = sum_c xT[c,t]*wv[c,d] (128-token
    stationary tiles), copied on ACT into the augmented-V buffer whose
    65th column of ones yields the softmax denominator during PV.
  - Attention in scores-transposed layout [keys, queries]: probsT =
    exp(K^T.T @ Q^T * 0.125) on ACT (bf16 out), causal diagonal-band
    masking alternating between gpsimd affine_select and DVE multiply by
    precomputed 0/1 masks. PV accumulates over key tiles; scores/PV
    software-pipelined with a 2-group lookahead.
  - Division by the softmax denominator folded into the PSUM->SBUF move.
  - Projection: partial outT = wpT.T @ attnT into a persistent buffer,
    6 large DMAs out as bf16 (768, 2048); host sums partials in fp32.

All SBUF tile pools are opened once, outside the For_i timing loop, so no
pool-boundary semaphore barriers fall inside an iteration; only the
per-phase PSUM pools (bank reuse: 6/8/4 banks) open inside.
"""

import numpy as np
import ml_dtypes

import concourse.bass as bass
import concourse.bacc as bacc
import concourse.tile as tile
from concourse import mybir
from concourse.bass_utils import run_bass_kernel_spmd

B, T, C, H = 2, 2048, 768, 12
D = C // H  # 64
ROPE_THETA = 10000.0
NCORES = 8
HPC = 3             # heads per core
NQF = 3             # packed q/k feature tiles: [q0|q1] [k0|k1] [q2|k2]
QB = 512            # query block (free dim of scores^T tiles)
KT = 128            # key tile (partition dim of scores^T tiles)

F32 = mybir.dt.float32
BF16 = mybir.dt.bfloat16
BF = ml_dtypes.bfloat16

# (feature-tile, half) of each head's q / k block in the packed layout
Q_POS = {0: (0, 0), 1: (0, 1), 2: (2, 0)}
K_POS = {0: (1, 0), 1: (1, 1), 2: (3, 0)}   # k2 DMA-copied to slot 3 lo-half


def _build_nc(t_len=T, loops=1, unroll=False, body=1):
    nc = bacc.Bacc("TRN2", target_bir_lowering=False, debug=False)

    xT_d = nc.dram_tensor("xT", [C, t_len], BF16, kind="ExternalInput")
    wqk_d = nc.dram_tensor("wqkT", [C, NQF * 128], BF16, kind="ExternalInput")
    wv_d = nc.dram_tensor("wvT", [C, HPC * D], BF16, kind="ExternalInput")
    wp_d = nc.dram_tensor("wpT", [HPC * D, C], BF16, kind="ExternalInput")
    cos_d = nc.dram_tensor("cosT", [128, t_len], F32, kind="ExternalInput")
    gsin_d = nc.dram_tensor("gsinT", [128, t_len], F32, kind="ExternalInput")
    p2t_d = nc.dram_tensor("p2t", [128, 128], BF16, kind="ExternalInput")
    msk_d = nc.dram_tensor("msk", [128, 4 * QB], BF16, kind="ExternalInput")
    outT_d = nc.dram_tensor("outT", [C, t_len], BF16, kind="ExternalOutput")

    with tile.TileContext(nc) as tc:
        _body(tc, t_len, xT_d, wqk_d, wv_d, wp_d, cos_d, gsin_d, p2t_d, msk_d,
              outT_d, loops=loops, unroll=unroll, body=body)
    nc.compile()
    return nc


def _body(tc, t_len, xT_d, wqk_d, wv_d, wp_d, cos_d, gsin_d, p2t_d, msk_d,
          outT_d, loops=1, unroll=False, body=1):
    T = t_len
    NCT = C // 128
    NKT = T // KT
    with (
        tc.tile_pool(name="singles", bufs=1) as singles,
        tc.tile_pool(name="sb_x", bufs=2) as sb_x,
        tc.tile_pool(name="sb_qs", bufs=3) as sb_qs,
        tc.tile_pool(name="sb_qc", bufs=3) as sb_qc,
        tc.tile_pool(name="sb_probs", bufs=6) as sb_probs,
        tc.tile_pool(name="sb_rcp", bufs=2) as sb_rcp,
    ):
        s = {}
        s["wqk"] = singles.tile([128, NCT, NQF * 128], BF16, tag="wqk", name="wqk")
        s["wv"] = singles.tile([128, NCT, HPC * D], BF16, tag="wv", name="wv")
        s["wp0"] = singles.tile([128, C], BF16, tag="wp0", name="wp0")
        s["wp1"] = singles.tile([64, C], BF16, tag="wp1", name="wp1")
        s["cosc"] = singles.tile([128, T], F32, tag="cosc", name="cosc")
        s["gsin"] = singles.tile([128, T], F32, tag="gsin", name="gsin")
        s["p2t"] = singles.tile([128, 128], BF16, tag="p2t", name="p2t")
        s["msk"] = singles.tile([128, 4, QB], BF16, tag="msk", name="msk")
        s["qkrot"] = singles.tile([128, NQF + 1, T], BF16, tag="qkrot", name="qkrot")
        s["va"] = singles.tile([128, NKT, HPC, D + 1], BF16, tag="va", name="va")
        s["at01"] = singles.tile([128, T], BF16, tag="at01", name="at01")
        s["at2"] = singles.tile([64, T], BF16, tag="at2", name="at2")
        s["outb"] = singles.tile([128, C // 128, T], BF16, tag="outb", name="outb")
        pools = dict(sb_x=sb_x, sb_qs=sb_qs, sb_qc=sb_qc, sb_probs=sb_probs,
                     sb_rcp=sb_rcp)

        if loops > 1 and unroll:
            for _ in range(loops * body):
                _compute(tc, t_len, s, pools, xT_d, wqk_d, wv_d, wp_d, cos_d,
                         gsin_d, p2t_d, msk_d, outT_d)
        elif loops > 1:
            with tc.For_i(0, loops, 1):
                for _ in range(body):
                    _compute(tc, t_len, s, pools, xT_d, wqk_d, wv_d, wp_d,
                             cos_d, gsin_d, p2t_d, msk_d, outT_d)
        else:
            for _ in range(body):
                _compute(tc, t_len, s, pools, xT_d, wqk_d, wv_d, wp_d, cos_d,
                         gsin_d, p2t_d, msk_d, outT_d)


def _compute(tc, t_len, s, pools, xT_d, wqk_d, wv_d, wp_d, cos_d, gsin_d,
             p2t_d, msk_d, outT_d):
    nc = tc.nc
    T = t_len
    NQB = T // QB       # 4 token/query blocks
    NKT = T // KT       # 16 key tiles
    NCT = C // 128      # 6 contraction tiles over channels
    JPB = QB // KT      # key tiles per token block (4)

    wqk, wv, wp0, wp1 = s["wqk"], s["wv"], s["wp0"], s["wp1"]
    cosc, gsin, p2t, msk = s["cosc"], s["gsin"], s["p2t"], s["msk"]
    qkrot, va, at01, at2, outb = (s["qkrot"], s["va"], s["at01"], s["at2"],
                                  s["outb"])
    sb_x, sb_qs, sb_qc = pools["sb_x"], pools["sb_qs"], pools["sb_qc"]
    sb_probs, sb_rcp = pools["sb_probs"], pools["sb_rcp"]

    wqk_v = wqk_d.ap().rearrange("(a p) f -> p a f", p=128)
    wv_v = wv_d.ap().rearrange("(a p) f -> p a f", p=128)
    xT_v = xT_d.ap().rearrange("(a p) t -> p a t", p=128)

    QB2 = 2 * QB  # 1024-token QKV blocks: half the matmul instructions
    NTB2 = T // QB2
    xtbs = []
    for tb in range(NTB2):
        xtb = sb_x.tile([128, NCT, QB2], BF16, tag="xtb", name=f"xtb{tb}")
        xtbs.append(xtb)
    # critical-path order: weights + x block 0 + rope tables first, the
    # remaining x blocks next, cold constants trailing
    nc.sync.dma_start(out=wqk, in_=wqk_v)
    nc.sync.dma_start(out=xtbs[0], in_=xT_v[:, :, 0:QB2])
    nc.sync.dma_start(out=cosc, in_=cos_d.ap())
    nc.sync.dma_start(out=gsin, in_=gsin_d.ap())
    nc.sync.dma_start(out=p2t, in_=p2t_d.ap())
    nc.sync.dma_start(out=wv, in_=wv_v)
    nc.sync.dma_start(out=xtbs[1], in_=xT_v[:, :, QB2:2 * QB2])
    nc.sync.dma_start(out=msk, in_=msk_d.ap().rearrange("p (a q) -> p a q", q=QB))
    nc.sync.dma_start(out=wp0, in_=wp_d.ap()[0:128, :])
    nc.sync.dma_start(out=wp1, in_=wp_d.ap()[128:192, :])

    # ones column of the augmented V tiles (softmax denominator)
    nc.vector.memset(va[:, :, :, D], 1.0)

    # ---- QKV projection + RoPE + direct V^T ------------------------------
    with (
        tc.tile_pool(name="ps_a", bufs=2, space="PSUM") as ps_a,
        tc.tile_pool(name="ps_r", bufs=1, space="PSUM") as ps_r,
        tc.tile_pool(name="ps_v", bufs=2, space="PSUM") as ps_v,
    ):
        # software pipeline: the rotate matmul for (tb, ft) is emitted one
        # acc-group later so the PE never waits on the DVE mul feeding it
        prev = []   # at most one (qs, qc, ft, tb) awaiting rotate

        def flush_rot():
            qs, qc, ft, tb = prev.pop(0)
            ts = slice(tb * QB2, (tb + 1) * QB2)
            rh = ps_r.tile([128, QB2], F32, tag="rh", name="rh")
            nc.tensor.matmul(rh, p2t, qs, start=True, stop=True)
            nc.vector.tensor_add(qkrot[:, ft, ts], qc, rh)
            if ft == 2:
                # scores need k2 on the same base partition as q2:
                # mirror the hi half into slot 3's lo half
                nc.sync.dma_start(out=qkrot[0:64, 3, ts],
                                  in_=qkrot[64:128, 2, ts])

        for tb in range(NTB2):
            ts = slice(tb * QB2, (tb + 1) * QB2)
            xtb = xtbs[tb]
            for ft in range(NQF):
                acc = ps_a.tile([128, QB2], F32, tag="acc", name="acc")
                for ct in range(NCT):
                    nc.tensor.matmul(
                        acc,
                        wqk[:, ct, ft * 128: (ft + 1) * 128],
                        xtb[:, ct, :],
                        start=(ct == 0),
                        stop=(ct == NCT - 1),
                    )
                if prev:
                    flush_rot()
                qs = sb_qs.tile([128, QB2], BF16, tag="qs", name="qs")
                nc.vector.tensor_mul(qs, acc, gsin[:, ts])
                qc = sb_qc.tile([128, QB2], BF16, tag="qc", name="qc")
                nc.vector.tensor_mul(qc, acc, cosc[:, ts])
                prev.append((qs, qc, ft, tb))
            # direct V^T for this token block (8 x 128-token tiles)
            for j in range(QB2 // KT):
                kt = tb * (QB2 // KT) + j
                vt = ps_v.tile([128, HPC * D], F32, tag="vt", name="vt")
                for ct in range(NCT):
                    nc.tensor.matmul(
                        vt,
                        xtb[:, ct, j * KT: (j + 1) * KT],
                        wv[:, ct, :],
                        start=(ct == 0),
                        stop=(ct == NCT - 1),
                    )
                if j == 0 and prev:
                    flush_rot()
                nc.scalar.copy(
                    va[:, kt, :, 0:D],
                    vt.rearrange("p (h d) -> p h d", h=HPC),
                )
        while prev:
            flush_rot()

    # ---- attention -------------------------------------------------------
    # Query blocks processed in pairs (qlo, qhi) = (2i, 2i+1), 1024 queries
    # wide. Key tiles kt < 4*(qlo+1) are causally valid for BOTH halves and
    # run as single 1024-wide scores/exp/PV ops; the remaining 4 key tiles
    # of qhi's diagonal band run 512-wide on the right half only.
    def qk_ap(pos, ts):
        ti, half = pos
        return qkrot[half * 64: half * 64 + 64, ti, ts]

    QB2 = 2 * QB
    with (
        tc.tile_pool(name="ps_sc", bufs=2, space="PSUM") as ps_sc,
        tc.tile_pool(name="ps_pv", bufs=2, space="PSUM") as ps_pv,
    ):
        pvs = {}     # (h, pair) -> pv tile [65, 1024]
        pend = []    # (h, pair, kind, ...) score tiles awaiting PV
        nmask = [0]  # running diag-mask count for engine alternation

        def emit_mask(ap, p):
            if nmask[0] % 2 == 0:
                nc.gpsimd.affine_select(
                    out=ap, in_=ap,
                    compare_op=mybir.AluOpType.is_ge,
                    fill=0.0, base=-p * KT,
                    pattern=[[1, QB]],
                    channel_multiplier=-1,
                )
            else:
                nc.vector.tensor_mul(ap, ap, msk[:, p, :])
            nmask[0] += 1

        def emit_pv(depth):
            while len(pend) > depth:
                h, pair, kind, probs, kts, nkt = pend.pop(0)
                pv = pvs[(h, pair)]
                if kind == "m":          # merged 1024-wide, single kt
                    kt = kts[0]
                    nc.tensor.matmul(
                        pv, va[:, kt, h, :], probs,
                        start=(kt == 0), stop=(kt == nkt - 1),
                        skip_group_check=True,
                    )
                else:                    # right-half 512-wide pair of kts
                    for j2, kt in enumerate(kts):
                        nc.tensor.matmul(
                            pv[:, QB:QB2], va[:, kt, h, :], probs[:, j2, :],
                            start=False, stop=(kt == nkt - 1),
                            skip_group_check=True,
                        )
                if kts[-1] == nkt - 1:
                    # pair done: fold denominator into the PSUM->SBUF move
                    rcp = sb_rcp.tile([1, QB2], F32, tag="rcp", name="rcp")
                    nc.vector.reciprocal(rcp, pv[64:65, :])
                    rcpb = sb_rcp.tile([64, QB2], F32, tag="rcpb", name="rcpb")
                    nc.gpsimd.partition_broadcast(rcpb, rcp)
                    qs_ = slice(pair * QB2, (pair + 1) * QB2)
                    if h == 0:
                        dst = at01[0:64, qs_]
                    elif h == 1:
                        dst = at01[64:128, qs_]
                    else:
                        dst = at2[:, qs_]
                    nc.vector.tensor_mul(dst, pv[0:64, :], rcpb)
                    del pvs[(h, pair)]

        scale = float(1.0 / np.sqrt(D))
        for h in range(HPC):
            for pair in range(NQB // 2):
                qlo, qhi = 2 * pair, 2 * pair + 1
                qs2 = slice(pair * QB2, (pair + 1) * QB2)
                qhi_s = slice(qhi * QB, (qhi + 1) * QB)
                nkt = (qhi + 1) * JPB
                mk = (qlo + 1) * JPB   # kts merged across both halves
                pvs[(h, pair)] = ps_pv.tile([65, QB2], F32, tag="pv",
                                            name=f"pv{h}_{pair}")
                for kt in range(mk):
                    sc = ps_sc.tile([128, QB2], F32, tag="sc", name="sc")
                    nc.tensor.matmul(
                        sc,
                        qk_ap(K_POS[h], slice(kt * KT, (kt + 1) * KT)),
                        qk_ap(Q_POS[h], qs2),
                        start=True, stop=True,
                    )
                    probs = sb_probs.tile([128, QB2], BF16, tag="probs",
                                          name="probs")
                    nc.scalar.activation(
                        probs, sc, mybir.ActivationFunctionType.Exp,
                        scale=scale,
                    )
                    p_lo = kt - qlo * JPB
                    if p_lo >= 0:        # qlo's diagonal band (left half)
                        emit_mask(probs[:, 0:QB], p_lo)
                    pend.append((h, pair, "m", probs, [kt], nkt))
                    emit_pv(1)
                for g in range((nkt - mk) // 2):   # qhi band, right half
                    k0, k1 = mk + 2 * g, mk + 2 * g + 1
                    sc = ps_sc.tile([128, QB2], F32, tag="sc", name="sc")
                    sc2 = sc.rearrange("p (a q) -> p a q", q=QB)
                    for j2, kt in enumerate((k0, k1)):
                        nc.tensor.matmul(
                            sc2[:, j2, :],
                            qk_ap(K_POS[h], slice(kt * KT, (kt + 1) * KT)),
                            qk_ap(Q_POS[h], qhi_s),
                            start=True, stop=True,
                        )
                    probs = sb_probs.tile([128, QB2], BF16, tag="probs",
                                          name="probs")
                    probs2 = probs.rearrange("p (a q) -> p a q", q=QB)
                    nc.scalar.activation(
                        probs2, sc2, mybir.ActivationFunctionType.Exp,
                        scale=scale,
                    )
                    for j2, kt in enumerate((k0, k1)):
                        emit_mask(probs2[:, j2, :], kt - qhi * JPB)
                    pend.append((h, pair, "b", probs2, [k0, k1], nkt))
                    emit_pv(1)
        emit_pv(0)

    # ---- output projection (partial over this core's 192 channels) -------
    with tc.tile_pool(name="ps_po", bufs=4, space="PSUM") as ps_po:
        k = 0
        for co in range(C // 128):
            for tb in range(NQB):
                ts = slice(tb * QB, (tb + 1) * QB)
                po = ps_po.tile([128, QB], F32, tag="po", name="po")
                nc.tensor.matmul(
                    po, wp0[:, co * 128: (co + 1) * 128],
                    at01[:, ts], start=True, stop=False,
                )
                nc.tensor.matmul(
                    po, wp1[:, co * 128: (co + 1) * 128],
                    at2[:, ts], start=False, stop=True,
                )
                ot = outb[:, co, ts]
                if k % 2 == 0:
                    nc.vector.tensor_copy(ot, po)
                else:
                    nc.scalar.copy(ot, po)
                k += 1
            nc.scalar.dma_start(
                out=outT_d.ap()[co * 128: (co + 1) * 128, :],
                in_=outb[:, co, :],
            )


_NC_CACHE = {}


def _get_nc():
    if "nc" not in _NC_CACHE:
        _NC_CACHE["nc"] = _build_nc()
    return _NC_CACHE["nc"]


def _host_consts(t_len=T):
    half = D // 2  # 32
    inv_freq = 1.0 / (ROPE_THETA ** (np.arange(0, D, 2, dtype=np.float32) / D))
    ang = np.arange(t_len, dtype=np.float32)[:, None] * inv_freq[None, :]
    sin = np.sin(ang).T.astype(np.float32)   # (32, T)
    cos = np.cos(ang).T.astype(np.float32)   # (32, T)
    cos64 = np.concatenate([cos, cos], axis=0)            # (64, T)
    gsin64 = np.concatenate([sin, -sin], axis=0)          # (64, T)
    cos128 = np.concatenate([cos64, cos64], axis=0)       # (128, T)
    gsin128 = np.concatenate([gsin64, gsin64], axis=0)    # (128, T)
    # plain half-swap permutation per 64-feature block:
    # out[m] = in[sigma(m)], sigma swaps 32-halves; p2t[sigma(m), m] = 1
    P64 = np.zeros((D, D), dtype=np.float32)
    P64[np.arange(half), np.arange(half) + half] = 1.0
    P64[np.arange(half) + half, np.arange(half)] = 1.0
    p2t = np.zeros((128, 128), dtype=np.float32)
    p2t[0:D, 0:D] = P64
    p2t[D:128, D:128] = P64
    # diag-band masks: msk[p][key, q] = 1 iff q - key >= 128*p
    k_idx = np.arange(KT)[:, None]
    q_idx = np.arange(QB)[None, :]
    msk = np.stack(
        [(q_idx - k_idx >= 128 * p).astype(np.float32) for p in range(4)],
        axis=1,
    ).reshape(KT, 4 * QB)
    return cos128, gsin128, p2t.astype(BF), msk.astype(BF)


def _pack_w(w_qkv, heads):
    """Pack this core's q/k rows into the (384, C) tile layout and v rows
    into (192, C)."""
    q = [w_qkv[0 * C + h * D: 0 * C + (h + 1) * D] for h in heads]
    kk = [w_qkv[1 * C + h * D: 1 * C + (h + 1) * D] for h in heads]
    v = [w_qkv[2 * C + h * D: 2 * C + (h + 1) * D] for h in heads]
    wqk = np.concatenate([q[0], q[1], kk[0], kk[1], q[2], kk[2]], axis=0)
    wv = np.concatenate(v, axis=0)
    return wqk, wv


def _make_in_maps(x, w_qkv, w_proj, t_len=T):
    cos128, gsin128, p2t, msk = _host_consts(t_len)
    in_maps = []
    for core in range(NCORES):
        b, hg = divmod(core, 4)
        heads = list(range(hg * HPC, (hg + 1) * HPC))
        wqk, wv = _pack_w(w_qkv, heads)
        cs = slice(hg * HPC * D, (hg + 1) * HPC * D)
        in_maps.append(
            {
                "xT": np.ascontiguousarray(x[b].T).astype(BF),
                "wqkT": np.ascontiguousarray(wqk.T).astype(BF),
                "wvT": np.ascontiguousarray(wv.T).astype(BF),
                "wpT": np.ascontiguousarray(w_proj[:, cs].T).astype(BF),
                "cosT": cos128, "gsinT": gsin128, "p2t": p2t, "msk": msk,
            }
        )
    return in_maps


def kernel(x, w_qkv, w_proj):
    x = np.asarray(x, dtype=np.float32)
    w_qkv = np.asarray(w_qkv, dtype=np.float32)
    w_proj = np.asarray(w_proj, dtype=np.float32)

    in_maps = _make_in_maps(x, w_qkv, w_proj)
    nc = _get_nc()
    res = run_bass_kernel_spmd(nc, in_maps, core_ids=list(range(NCORES)))
    out = np.zeros((B, T, C), dtype=np.float32)
    for core in range(NCORES):
        b = core // 4
        out[b] += res.results[core]["outT"].T.astype(np.float32)
    return out
